# revision 6
# baseline (speedup 1.0000x reference)
"""Trainium2 Bass kernel for unscaled Luong dot-product attention.

Problem: B=16, Tq=Tk=D=1024, fp32.
    scores = Q @ E^T ; weights = softmax(scores, -1) ; out = weights @ E

Sharding: data-parallel over batch — each of the 8 NeuronCores processes
2 batches end-to-end; no cross-core communication.

Per-core pipeline (per batch, per 128-row q-block):
  1. PE-transpose Q and E tiles (fp32) into [D-part, T-free] layout; the
     PSUM->SBUF copies round the operands to float32r (tf32-like) plus a
     float32r residual term (3xTF32 split) so the QK^T matmul runs at the
     full PE rate with ~fp32-grade accuracy.
  2. bmm1: scores[q,k] accumulated over 3 passes x 8 d-chunks in PSUM.
  3. Softmax along the free dim: DVE reduce_max (negated) -> one ACT exp
     with per-partition bias and fused row-sum accumulation -> DVE
     reciprocal. Normalization is folded into the output copy.
  4. PE-transpose the weights block, round to float32r, and run bmm2
     against E kept in natural [k,d] layout (single fp32r pass).
"""

import numpy as np

import concourse.bass as bass
import concourse.tile as tile
from concourse import bacc, mybir
from concourse.masks import make_identity

P = 128
B_PER_CORE = 2
T = 1024  # Tq = Tk
D = 1024
NC_CHUNKS = T // P  # 8 k-chunks / q-blocks
ND_CHUNKS = D // P  # 8 d-chunks
F32 = mybir.dt.float32
F32R = mybir.dt.float32r


def _transpose_block_group(nc, trans_psum, ident, src_fn, dst_r, dst_l, n_blocks=4):
    """Transpose `n_blocks` [128,128] fp32 SBUF blocks through one PSUM bank,
    then round the packed result into float32r `dst_r` and (optionally) the
    residual into float32r `dst_l` (3xTF32 split). src_fn(j) -> source AP."""
    tp = trans_psum.tile([P, n_blocks * P], F32)
    for j in range(n_blocks):
        nc.tensor.transpose(tp[:, j * P : (j + 1) * P], src_fn(j), ident)
    nc.vector.tensor_copy(dst_r, tp[:])
    if dst_l is not None:
        nc.vector.tensor_tensor(dst_l, tp[:], dst_r, mybir.AluOpType.subtract)


def build_nc(reps: int = 1):
    nc = bacc.Bacc("TRN2", target_bir_lowering=False, debug=False)
    q_dram = nc.dram_tensor("q", [B_PER_CORE, T, D], F32, kind="ExternalInput").ap()
    e_dram = nc.dram_tensor("e", [B_PER_CORE, T, D], F32, kind="ExternalInput").ap()
    o_dram = nc.dram_tensor("o", [B_PER_CORE, T, D], F32, kind="ExternalOutput").ap()

    with tile.TileContext(nc) as tc:
        with (
            tc.tile_pool(name="const", bufs=1) as const_pool,
            tc.tile_pool(name="e_nat", bufs=1) as e_nat_pool,
            tc.tile_pool(name="e_r", bufs=1) as e_r_pool,
            tc.tile_pool(name="etr", bufs=1) as etr_pool,
            tc.tile_pool(name="etl", bufs=1) as etl_pool,
            tc.tile_pool(name="qstage", bufs=2) as q_pool,
            tc.tile_pool(name="qt", bufs=2) as qt_pool,
            tc.tile_pool(name="w", bufs=2) as w_pool,
            tc.tile_pool(name="wt", bufs=2) as wt_pool,
            tc.tile_pool(name="ctx", bufs=2) as ctx_pool,
            tc.tile_pool(name="stat", bufs=4) as stat_pool,
            tc.tile_pool(name="sc_ps", bufs=2, space="PSUM") as sc_psum,
            tc.tile_pool(name="ctx_ps", bufs=1, space="PSUM") as ctx_psum,
            tc.tile_pool(name="tr_ps", bufs=2, space="PSUM") as trans_psum,
        ):
            ident = const_pool.tile([P, P], F32)
            make_identity(nc, ident[:])

            for b in [b for _ in range(reps) for b in range(B_PER_CORE)]:
                # ---- E setup: natural layout + f32r copy + 3xTF32 ET ----
                e_nat = e_nat_pool.tile([P, NC_CHUNKS, D], F32)
                for kc in range(NC_CHUNKS):
                    nc.sync.dma_start(
                        e_nat[:, kc, :], e_dram[b, kc * P : (kc + 1) * P, :]
                    )
                e_r = e_r_pool.tile([P, NC_CHUNKS, D], F32R)
                for kc in range(NC_CHUNKS):
                    nc.vector.tensor_copy(e_r[:, kc, :], e_nat[:, kc, :])

                # ET[d, k]: etr/etl [128(d), dc, T(k)]
                etr = etr_pool.tile([P, ND_CHUNKS, T], F32R)
                etl = etl_pool.tile([P, ND_CHUNKS, T], F32R)
                for dc in range(ND_CHUNKS):
                    for g in range(NC_CHUNKS // 4):
                        _transpose_block_group(
                            nc,
                            trans_psum,
                            ident[:],
                            lambda j, dc=dc, g=g: e_nat[
                                :, g * 4 + j, dc * P : (dc + 1) * P
                            ],
                            etr[:, dc, g * 512 : (g + 1) * 512],
                            etl[:, dc, g * 512 : (g + 1) * 512],
                        )

                for qb in range(NC_CHUNKS):
                    # ---- Q block: stage + transpose + 3xTF32 split ----
                    qstage = q_pool.tile([P, D], F32)
                    nc.sync.dma_start(qstage[:], q_dram[b, qb * P : (qb + 1) * P, :])
                    qtr = qt_pool.tile([P, ND_CHUNKS, P], F32R, tag="qtr")
                    qtl = qt_pool.tile([P, ND_CHUNKS, P], F32R, tag="qtl")
                    for g in range(ND_CHUNKS // 4):
                        _transpose_block_group(
                            nc,
                            trans_psum,
                            ident[:],
                            lambda j, g=g: qstage[:, (g * 4 + j) * P : (g * 4 + j + 1) * P],
                            qtr[:, g * 4 : (g + 1) * 4, :],
                            qtl[:, g * 4 : (g + 1) * 4, :],
                        )

                    # ---- bmm1: scores[q,k], 3 passes x 8 d-chunks ----
                    sc_ps = sc_psum.tile([P, T], F32)
                    pairs = [(qtr, etr), (qtl, etr), (qtr, etl)]
                    n_acc = len(pairs) * ND_CHUNKS
                    i = 0
                    for lhs, rhs in pairs:
                        for dc in range(ND_CHUNKS):
                            for kh in range(2):
                                nc.tensor.matmul(
                                    sc_ps[:, kh * 512 : (kh + 1) * 512],
                                    lhs[:, dc, :],
                                    rhs[:, dc, kh * 512 : (kh + 1) * 512],
                                    start=(i == 0),
                                    stop=(i == n_acc - 1),
                                )
                            i += 1

                    # ---- softmax along free dim ----
                    negmax = stat_pool.tile([P, 1], F32, tag="negmax")
                    nc.vector.tensor_reduce(
                        out=negmax[:],
                        in_=sc_ps[:],
                        op=mybir.AluOpType.max,
                        axis=mybir.AxisListType.X,
                        negate=True,
                    )
                    w_sb = w_pool.tile([P, T], F32)
                    ssum = stat_pool.tile([P, 1], F32, tag="ssum")
                    nc.scalar.activation(
                        w_sb[:],
                        sc_ps[:],
                        mybir.ActivationFunctionType.Exp,
                        bias=negmax[:],
                        accum_out=ssum[:],
                    )
                    recip = stat_pool.tile([P, 1], F32, tag="recip")
                    nc.vector.reciprocal(recip[:], ssum[:])

                    # ---- transpose W -> WT (f32r) ----
                    wt = wt_pool.tile([P, NC_CHUNKS, P], F32R)
                    for g in range(NC_CHUNKS // 4):
                        _transpose_block_group(
                            nc,
                            trans_psum,
                            ident[:],
                            lambda j, g=g: w_sb[:, (g * 4 + j) * P : (g * 4 + j + 1) * P],
                            wt[:, g * 4 : (g + 1) * 4, :],
                            None,
                        )

                    # ---- bmm2: ctx[q,d] = WT.T @ E ----
                    ctx_ps = ctx_psum.tile([P, D], F32)
                    for kc in range(NC_CHUNKS):
                        for dh in range(2):
                            nc.tensor.matmul(
                                ctx_ps[:, dh * 512 : (dh + 1) * 512],
                                wt[:, kc, :],
                                e_r[:, kc, dh * 512 : (dh + 1) * 512],
                                start=(kc == 0),
                                stop=(kc == NC_CHUNKS - 1),
                            )

                    ctx_sb = ctx_pool.tile([P, D], F32)
                    nc.vector.tensor_scalar_mul(ctx_sb[:], ctx_ps[:], recip[:])
                    nc.sync.dma_start(o_dram[b, qb * P : (qb + 1) * P, :], ctx_sb[:])

    nc.compile()
    return nc


_NC_CACHE = None


def _get_nc():
    global _NC_CACHE
    if _NC_CACHE is None:
        _NC_CACHE = build_nc()
    return _NC_CACHE


def kernel(decoder_hidden: np.ndarray, encoder_outputs: np.ndarray) -> np.ndarray:
    from concourse import bass_utils

    dh = np.ascontiguousarray(np.asarray(decoder_hidden, dtype=np.float32))
    eo = np.ascontiguousarray(np.asarray(encoder_outputs, dtype=np.float32))
    assert dh.shape == (16, T, D) and eo.shape == (16, T, D)

    nc = _get_nc()
    in_maps = [
        {
            "q": dh[i * B_PER_CORE : (i + 1) * B_PER_CORE],
            "e": eo[i * B_PER_CORE : (i + 1) * B_PER_CORE],
        }
        for i in range(8)
    ]
    res = bass_utils.run_bass_kernel_spmd(nc, in_maps, core_ids=list(range(8)))
    return np.concatenate([r["o"] for r in res.results], axis=0)
